# revision 64
# baseline (speedup 1.0000x reference)
"""GAT layer (nn_GATLayer) Trainium2 Bass kernel.

Reference math:
    NF = x @ W.T + b                     # [N, 256] -> heads [N, 8, 32]
    lp[i,h] = sum_d NF[i,h,d] * a[h,d];  lc[j,h] = sum_d NF[j,h,d] * a[h,32+d]
    logits[i,j,h] = leaky_relu(lp+lc, 0.2) masked to 0 where adj==0
    out[i,h,:] = softmax_j(logits) @ NF[:,h,:]

Identities (adj in {0,1}):
    exp(leaky_relu(z, .2)) = exp(.2 z) * max(exp(.8 z), 1)
    u'[j,i,h] = adj[j,i] * max(A8[i,h]*B8[j,h], 1)   # one masked stream/head
    num[i,h,c] = S[c] - M3[i,c] + A2[i,h] * (u' @ (B2*NF)_h)[i,c]
    Z[i,h]     = N - deg[i]     + A2[i,h] * (u' @ B2_h)[i]
    out = num / Z
    with A8=exp(.8 lp), B8=exp(.8 lc), A2=exp(.2 lp), B2=exp(.2 lc),
    S = colsum(NF), M3 = adj @ NF, deg = adj @ 1.

Layout: 388-column "bank" layout, 4 banks of 97 cols each; bank b holds
heads (2b, 2b+1) at col offsets 0 and 64 (33 cols each: 32 feats + z/one
col), cols 33..63 zero. PSUM: 4 pacc banks [97,512] (u'@B2NF streams at
partition offsets 0/64) + 4 psh banks [97,512] (M3 + deg via ones-cols,
aligned with pacc) = 8 banks, single pass over j (32 chunks of 128).
Heads 6,7 use the rr-route: ACT computes rr=relu(A8*B8-1); the missing
"+1" rides as a 13th PE stream (stationary = aggW bank3, moving = adjT)
accumulating adj@[B2NF|B2] into pacc bank3.
"""

import numpy as np
import ml_dtypes

import concourse.bass as bass
import concourse.bacc as bacc
import concourse.tile as tile
from concourse import mybir
from concourse.bass_utils import run_bass_kernel_spmd
from concourse.masks import make_identity

N_CORES = 8
N = 4096
IN_FEAT = 256
OUT_FEAT = 256
H = 8
D = 32
R = N // N_CORES          # rows (parents) per core = 512
JC = N // 128             # j-chunks of 128 = 32
WK = OUT_FEAT + H         # NF cols + lc cols in the dense weight = 264
BANKW = 97                # per-bank col width in the 388 layout
LAYW = 4 * BANKW          # 388

FP = mybir.dt.float32
FR = mybir.dt.float32r
BF = mybir.dt.bfloat16
AF = mybir.ActivationFunctionType
ALU = mybir.AluOpType


def _col97(h):
    return BANKW * (h // 2) + 64 * (h % 2)


def build_program():
    nc = bacc.Bacc("TRN2", target_bir_lowering=False, debug=False,
                   num_devices=N_CORES)

    # host-packed inputs
    xk_in = nc.dram_tensor("xk", [128, JC * 256], FR, kind="ExternalInput").ap()
    xr_in = nc.dram_tensor("xr", [128, 4 * 256 + 2], FP,
                           kind="ExternalInput").ap()
    wk_in = nc.dram_tensor("wkk", [128, 2 * WK], FR, kind="ExternalInput").ap()
    wk2_in = nc.dram_tensor("wk2", [1, WK + 129], FR, kind="ExternalInput").ap()
    wap_in = nc.dram_tensor("wap", [128, 2 * H], FP, kind="ExternalInput").ap()
    wap2_in = nc.dram_tensor("wap2", [1, H], FP, kind="ExternalInput").ap()
    adj_in = nc.dram_tensor("adjc", [128, JC * R], BF, kind="ExternalInput").ap()
    s97_in = nc.dram_tensor("sel97h", [2, BANKW], FR, kind="ExternalInput").ap()
    sZ_in = nc.dram_tensor("selZh", [BANKW, 2], FP, kind="ExternalInput").ap()
    wsb_in = nc.dram_tensor("wsb", [128, 2 * OUT_FEAT], BF,
                            kind="ExternalInput").ap()
    ws2_in = nc.dram_tensor("ws2", [1, OUT_FEAT], BF, kind="ExternalInput").ap()
    xsb_in = nc.dram_tensor("xsb", [128, 2], BF, kind="ExternalInput").ap()
    outB = nc.dram_tensor("outB", [LAYW, R], FP, kind="ExternalOutput").ap()

    with tile.TileContext(nc) as tc:
        from contextlib import ExitStack
        with ExitStack() as top:
            consts = top.enter_context(tc.tile_pool(name="consts", bufs=1))
            persist = top.enter_context(tc.tile_pool(name="persist", bufs=1))
            atpool = top.enter_context(tc.tile_pool(name="at", bufs=3))

            ident = consts.tile([128, 128], FP)
            make_identity(nc, ident[:])
            wk2full = consts.tile([1, WK + 129], FR)
            negone = consts.tile([128, 1], FP)
            nc.vector.memset(negone[:], -1.0)
            sel97 = consts.tile([2, BANKW], FR)
            selZ = consts.tile([BANKW, 2], FP)

            wkk = consts.tile([128, 2, WK], FR)
            wap = consts.tile([128, 2, H], FP)
            wsb = consts.tile([128, 2, OUT_FEAT], BF)
            ws2 = consts.tile([1, OUT_FEAT], BF)
            xsb = consts.tile([128, 2], BF)
            nb1 = consts.tile([1, 1], BF)
            nc.vector.memset(nb1[:], float(N))
            onesrowF = consts.tile([1, 128], FP)
            nc.vector.memset(onesrowF[:], 1.0)
            nc.sync.dma_start(out=wap.rearrange("p a b -> p (a b)"),
                              in_=wap_in[:])
            wap2 = consts.tile([1, H], FP)

            nc.sync.dma_start(out=wap2[:], in_=wap2_in[:])

            # persistent SBUF
            shW = persist.tile([128, JC, LAYW], BF)
            aggW = persist.tile([128, JC, LAYW], BF)
            b8c = persist.tile([128, JC, H], FP)
            b2t = persist.tile([128, JC, H], BF)
            a8rep = persist.tile([128, H, R], BF)
            a2rep = persist.tile([128, H, R], FP)
            lpT = persist.tile([H, R], FP)
            a8Tb = persist.tile([H, R], BF)
            a2T = persist.tile([H, R], FP)
            a8st = persist.tile([1, H * R], BF)
            a2st = persist.tile([1, H * R], FP)
            scol = persist.tile([128, 4], FP)
            numT = persist.tile([128, 4, R], FP)
            t1s = persist.tile([128, 4, R], FP)
            tmpP = persist.tile([BANKW, 4, R], FP)
            pzS = persist.tile([BANKW, 4, R], FP)
            outTs = persist.tile([128, 4, R], FP)
            rzv = persist.tile([2, 4, R], FR)

            # shW gaps must be finite: psh gap partitions feed numT's gap
            # rows which the one-hot Z-extraction matmul contracts over
            # (0 * NaN = NaN). aggW gaps only feed pacc partitions 33-63,
            # which are never read, so they stay uninitialized.
            shw_gap = bass.AP(tensor=shW.tensor, offset=shW.offset + 33,
                              ap=[shW.ap[0], [LAYW, JC], [BANKW, 4], [1, 31]])
            nc.vector.memset(shw_gap, 0.0)
            shw_one = bass.AP(tensor=shW.tensor, offset=shW.offset + 32,
                              ap=[shW.ap[0], [LAYW, JC], [BANKW, 4], [64, 2]])
            nc.vector.memset(shw_one, 1.0)
            nc.vector.memset(t1s[32:64, :, :], 0.0)
            nc.vector.memset(t1s[96:128, :, :], 0.0)

            # ---- Phase 0 ----
            with ExitStack() as ph0:
                xpool = ph0.enter_context(tc.tile_pool(name="xp", bufs=1))
                ps0 = ph0.enter_context(
                    tc.tile_pool(name="ps0", bufs=1, space="PSUM"))
                psnf = ph0.enter_context(
                    tc.tile_pool(name="psnf", bufs=3, space="PSUM"))

                xr = xpool.tile([128, 4, 2, 128], FP)
                xrf = xr.rearrange("p a b c -> p (a b c)")
                nc.sync.dma_start(out=xrf[:], in_=xr_in[:, 0:1024])
                nc.sync.dma_start(out=wsb.rearrange("p a b -> p (a b)"),
                                  in_=wsb_in[:])
                nc.sync.dma_start(out=ws2[:], in_=ws2_in[:])
                nc.sync.dma_start(out=xsb[:], in_=xsb_in[:])
                nc.sync.dma_start(out=wkk.rearrange("p a b -> p (a b)"),
                                  in_=wk_in[:])
                nc.sync.dma_start(out=wk2full[:], in_=wk2_in[:])
                wk2v = wk2full[0:1, 0:WK]
                onesrow = wk2full[0:1, WK:WK + 128]
                none1 = wk2full[0:1, WK + 128:WK + 129]
                nc.sync.dma_start(out=sel97[:], in_=s97_in[:])
                nc.sync.dma_start(out=selZ[:], in_=sZ_in[:])
                xk = xpool.tile([128, JC, 2, 128], FR)
                xkf = xk.rearrange("p a b c -> p (a b c)")
                for q in range(4):
                    nc.sync.dma_start(out=xkf[:, q * 2048:(q + 1) * 2048],
                                      in_=xk_in[:, q * 2048:(q + 1) * 2048])

                # prefetch first two adjacency blocks during phase 0
                at_tiles = {}
                for blk in (0, 1):
                    t = atpool.tile([128, 4, R], BF, name="at4")
                    nc.sync.dma_start(
                        out=t.rearrange("p a b -> p (a b)"),
                        in_=adj_in[:, blk * 4 * R:(blk + 1) * 4 * R])
                    at_tiles[blk] = t

                # lp chain for own rows
                for rb in range(4):
                    plp = ps0.tile([128, H], FP, space="PSUM", tag="plp")
                    nc.tensor.matmul(plp[:], xr[:, rb, 0, :], wap[:, 0, :],
                                     start=True, stop=False)
                    nc.tensor.matmul(plp[:], xr[:, rb, 1, :], wap[:, 1, :],
                                     start=False, stop=False)
                    nc.tensor.matmul(plp[:], onesrowF[:], wap2[:],
                                     start=False, stop=True)
                    lps = xpool.tile([128, H], FP, name="lps")
                    nc.scalar.copy(lps[:], plp[:])
                    plpT = ps0.tile([H, 128], FP, space="PSUM", tag="plp")
                    nc.tensor.transpose(plpT[:], lps[:], ident[:])
                    nc.scalar.copy(lpT[:, rb * 128:(rb + 1) * 128], plpT[:])

                nc.scalar.activation(a8Tb[:], lpT[:], AF.Exp, bias=0.0,
                                     scale=0.8)
                nc.scalar.activation(a2T[:], lpT[:], AF.Exp, bias=0.0,
                                     scale=0.2)
                # collapse the 8 rows into one partition line, then
                # partition_broadcast (Pool, SBUF-only) per head
                nc.sync.dma_start(out=a8st[:], in_=a8Tb[:])
                nc.sync.dma_start(out=a2st[:], in_=a2T[:])

                def rep_pair(k):
                    if k == 0:
                        nc.gpsimd.partition_broadcast(
                            a8rep.rearrange("p h r -> p (h r)"), a8st[0:1, :])
                    elif k == 1:
                        nc.gpsimd.partition_broadcast(
                            a2rep.rearrange("p h r -> p (h r)"), a2st[0:1, :])

                # S columns: scolP[p, b] over 3 k-chunks, rows 0-31 & 64-95
                scolP = ps0.tile([128, 4], FP, space="PSUM", tag="plp")
                for b in range(4):
                    for e in range(2):
                        cols = slice(64 * b + 32 * e, 64 * b + 32 * e + 32)
                        out_ap = scolP[64 * e:64 * e + 32, b:b + 1]
                        nc.tensor.matmul(out_ap, wsb[:, 0, cols],
                                         xsb[:, 0:1], start=True, stop=False)
                        nc.tensor.matmul(out_ap, wsb[:, 1, cols],
                                         xsb[:, 1:2], start=False, stop=False)
                        nc.tensor.matmul(out_ap, ws2[0:1, cols], nb1[:],
                                         start=False, stop=True)
                nc.vector.memset(scolP[32:64, :], float(N))
                nc.vector.memset(scolP[96:128, :], float(N))
                nc.scalar.copy(scol[:], scolP[:])

                # aggW sweep: aggW[:, nb, 33-blocks] = shW * b2
                def agg_sweep(nb, eng):
                    sv = bass.AP(tensor=shW.tensor,
                                 offset=shW.offset + nb * LAYW,
                                 ap=[shW.ap[0], [BANKW, 4], [64, 2], [1, 33]])
                    av = bass.AP(tensor=aggW.tensor,
                                 offset=aggW.offset + nb * LAYW,
                                 ap=[aggW.ap[0], [BANKW, 4], [64, 2], [1, 33]])
                    bv = bass.AP(tensor=b2t.tensor, offset=b2t.offset + nb * H,
                                 ap=[b2t.ap[0], [2, 4], [1, 2], [0, 33]])
                    eng.tensor_mul(av, sv, bv)

                # NF loop: pairs of chunks; pnf2 = [128, 2, 512] (2 banks)
                PA_ENG = [nc.vector, nc.gpsimd, nc.scalar, nc.vector,
                          nc.gpsimd, nc.scalar, nc.vector, nc.gpsimd]
                for t in range(JC // 2):
                    pnf2 = psnf.tile([128, 2, 512], FP, space="PSUM",
                                     tag="pnf2")
                    for e in range(2):
                        nb = 2 * t + e
                        pnf = pnf2[:, e, 0:WK]
                        nc.tensor.matmul(pnf, xk[:, nb, 0, :], wkk[:, 0, :],
                                         start=True, stop=False)
                        nc.tensor.matmul(pnf, xk[:, nb, 1, :], wkk[:, 1, :],
                                         start=False, stop=False)
                        nc.tensor.matmul(pnf, onesrow[:], wk2v[:],
                                         start=False, stop=True)
                    for e in range(2):
                        nb = 2 * t + e
                        dst = bass.AP(
                            tensor=shW.tensor,
                            offset=shW.offset + nb * LAYW,
                            ap=[shW.ap[0], [BANKW, 4], [64, 2], [1, 32]])
                        srcv = bass.AP(
                            tensor=pnf2.tensor,
                            offset=pnf2.offset + e * 512,
                            ap=[pnf2.ap[0], [64, 4], [32, 2], [1, 32]])
                        if e == 1:
                            nc.scalar.copy(dst, srcv)
                        else:
                            nc.vector.tensor_copy(dst, srcv)
                    lcsrc = bass.AP(tensor=pnf2.tensor,
                                    offset=pnf2.offset + 256,
                                    ap=[pnf2.ap[0], [512, 2], [1, H]])
                    nb0 = 2 * t
                    nc.scalar.activation(
                        b8c[:, nb0:nb0 + 2, :].rearrange("p a b -> p (a b)"),
                        lcsrc, AF.Exp, bias=0.0, scale=0.8)
                    nc.scalar.activation(
                        b2t[:, nb0:nb0 + 2, :].rearrange("p a b -> p (a b)"),
                        lcsrc, AF.Exp, bias=0.0, scale=0.2)
                    if nb0 < 19:
                        agg_sweep(nb0, nc.vector if nb0 % 2 == 0
                                  else nc.gpsimd)
                    if nb0 + 1 < 19:
                        agg_sweep(nb0 + 1, nc.vector if (nb0 + 1) % 2 == 0
                                  else nc.gpsimd)
                    if t == 0:
                        rep_pair(0)
                        rep_pair(1)

            # ---- Phase 1: single-pass j-loop ----
            with ExitStack() as ph1:
                acc = ph1.enter_context(
                    tc.tile_pool(name="acc", bufs=1, space="PSUM"))
                work = ph1.enter_context(tc.tile_pool(name="work", bufs=2))
                pacc = [acc.tile([BANKW, R], FP, space="PSUM",
                                 name=f"pacc{b}") for b in range(4)]
                psh = [acc.tile([BANKW, R], FP, space="PSUM",
                                name=f"psh{b}") for b in range(4)]
                for jc in range(JC):
                    blk = jc // 4
                    if jc % 4 == 0 and blk + 2 < JC // 4:
                        t = atpool.tile([128, 4, R], BF, name="at4")
                        nc.sync.dma_start(
                            out=t.rearrange("p a b -> p (a b)"),
                            in_=adj_in[:, (blk + 2) * 4 * R:(blk + 3) * 4 * R])
                        at_tiles[blk + 2] = t
                    at4 = at_tiles[blk]
                    at = at4[:, jc % 4, :]
                    tb = work.tile([128, H, R], BF, name="tb")
                    # tb h0,h1 on DVE; h2-4 on Pool; h5 chain + h6,h7 rr on ACT
                    for h in (0, 1):
                        nc.vector.tensor_scalar(tb[:, h, :], a8rep[:, h, :],
                                                b8c[:, jc, h:h + 1], 1.0,
                                                ALU.mult, ALU.max)
                    for h in (2, 3, 4):
                        nc.gpsimd.tensor_scalar(tb[:, h, :], a8rep[:, h, :],
                                                b8c[:, jc, h:h + 1], 1.0,
                                                ALU.mult, ALU.max)
                    rr5 = work.tile([128, R], BF, name="rr5")
                    nc.scalar.activation(rr5[:], a8rep[:, 5, :], AF.Relu,
                                         bias=negone[:],
                                         scale=b8c[:, jc, 5:6])
                    nc.scalar.activation(tb[:, 5, :], rr5[:], AF.Copy,
                                         bias=1.0, scale=1.0)
                    for h in (6, 7):
                        nc.scalar.activation(tb[:, h, :], a8rep[:, h, :],
                                             AF.Relu, bias=negone[:],
                                             scale=b8c[:, jc, h:h + 1])
                    # merged mask: s1 = tb * at (broadcast over head dim)
                    s1 = work.tile([128, H, R], BF, name="s1")
                    atb = bass.AP(tensor=at.tensor, offset=at.offset,
                                  ap=[at.ap[0], [0, H], at.ap[1]])
                    nc.vector.tensor_mul(s1[:], tb[:], atb)
                    if jc % 2 == 0 and 19 + jc // 2 < JC:
                        agg_sweep(19 + jc // 2, nc.vector)
                    # PE streams: 4 psh + aggX(bank3) + 8 s1
                    st = (jc == 0)
                    sp = (jc == JC - 1)
                    for b in range(4):
                        nc.tensor.matmul(
                            psh[b][:], shW[:, jc, b * BANKW:(b + 1) * BANKW],
                            at[:], start=st, stop=sp, skip_group_check=True)
                    if not sp:
                        nc.tensor.matmul(
                            pacc[3][:], aggW[:, jc, 3 * BANKW:4 * BANKW],
                            at[:], start=st, stop=False,
                            skip_group_check=True)
                    for h in range(H):
                        b, off = h // 2, 64 * (h % 2)
                        rrh = h >= 6
                        nc.tensor.matmul(
                            pacc[b][off:off + 33, :],
                            aggW[:, jc, _col97(h):_col97(h) + 33],
                            s1[:, h, :],
                            start=(st and not rrh),
                            stop=(sp and not rrh),
                            skip_group_check=True)
                    if sp:
                        nc.tensor.matmul(
                            pacc[3][:], aggW[:, jc, 3 * BANKW:4 * BANKW],
                            at[:], start=False, stop=True,
                            skip_group_check=True)

                # epilogue per bank, fully pipelined: t1 = pacc*a2;
                # numT = (t1+scol)-psh; rz = 1/Z; pz = bcast(rz) reusing the
                # psh bank; out = numT*pz; DMA out.
                pstride = numT.ap[0][0]
                with nc.allow_low_precision(reason="1/Z in f32r"):
                    for b in range(4):
                        # odd-head slice staged via Act, multiplied on Pool
                        nc.scalar.copy(tmpP[64:97, b, :], pacc[b][64:97, :])
                        nc.gpsimd.tensor_mul(t1s[64:97, b, :],
                                             tmpP[64:97, b, :],
                                             a2rep[64:97, 2 * b + 1, :])
                        nc.vector.tensor_mul(t1s[0:33, b, :],
                                             pacc[b][0:33, :],
                                             a2rep[0:33, 2 * b, :])
                        nc.vector.scalar_tensor_tensor(
                            numT[0:BANKW, b, :], t1s[0:BANKW, b, :],
                            scol[0:BANKW, b:b + 1], psh[b][:],
                            ALU.add, ALU.subtract)
                        nc.tensor.matmul(psh[b][0:2, :], selZ[:],
                                         numT[0:BANKW, b, :],
                                         start=True, stop=True,
                                         skip_group_check=True)
                        nc.vector.reciprocal(rzv[:, b, :], psh[b][0:2, :])
                        nc.tensor.matmul(psh[b][:], sel97[:], rzv[:, b, :],
                                         start=True, stop=True,
                                         skip_group_check=True)
                        nc.scalar.copy(pzS[0:BANKW, b, :], psh[b][:])
                        nc.gpsimd.tensor_mul(outTs[0:BANKW, b, :],
                                             numT[0:BANKW, b, :],
                                             pzS[0:BANKW, b, :])
                        nc.sync.dma_start(
                            out=outB[b * BANKW:(b + 1) * BANKW, :],
                            in_=outTs[0:BANKW, b, :])

    nc.compile()
    return nc


_PROGRAM_CACHE = {}


def kernel(x, W, b, a, adj_matrix):
    x = np.asarray(x, dtype=np.float32)
    W = np.asarray(W, dtype=np.float32)
    b = np.asarray(b, dtype=np.float32)
    a = np.asarray(a, dtype=np.float32)
    adj = np.asarray(adj_matrix, dtype=np.float32)

    wTa = np.vstack([W.T, b[None, :]])                      # [257, 256]
    Ap = np.zeros((OUT_FEAT, H), np.float32)
    Ac = np.zeros((OUT_FEAT, H), np.float32)
    for h in range(H):
        Ap[h * D:(h + 1) * D, h] = a[h, :D]
        Ac[h * D:(h + 1) * D, h] = a[h, D:]
    WAp = wTa @ Ap                                          # [257, 8]
    WAc = wTa @ Ac
    wk_full = np.hstack([wTa, WAc])                         # [257, 264]
    wkk_host = np.ascontiguousarray(
        wk_full[0:256].reshape(2, 128, WK).transpose(1, 0, 2).reshape(128, -1))
    wsb_host = np.ascontiguousarray(
        wk_full[0:256, 0:256].reshape(2, 128, 256).transpose(1, 0, 2)
        .reshape(128, -1)).astype(ml_dtypes.bfloat16)
    ws2_host = wk_full[256:257, 0:256].astype(ml_dtypes.bfloat16)
    wk2_host = np.empty((1, WK + 129), np.float32)
    wk2_host[0, 0:WK] = wk_full[256]
    wk2_host[0, WK:WK + 128] = 1.0
    wk2_host[0, WK + 128] = float(N)
    wap_host = np.ascontiguousarray(
        WAp[0:256].reshape(2, 128, H).transpose(1, 0, 2).reshape(128, -1))
    wap2_host = np.ascontiguousarray(WAp[256:257])

    # xk[p, jc*256 + k*128 + c] = x[jc*128+c, k*128+p]
    xk_host = np.ascontiguousarray(
        x.reshape(JC, 128, 2, 128).transpose(3, 0, 2, 1).reshape(128, -1))
    xsum = x.sum(axis=0, dtype=np.float64).astype(np.float32)  # [256]
    xsb_host = np.empty((128, 2), np.float32)
    xsb_host[:, 0] = xsum[0:128]
    xsb_host[:, 1] = xsum[128:256]
    xsb_host = xsb_host.astype(ml_dtypes.bfloat16)
    sel97_host = np.zeros((2, BANKW), np.float32)
    sel97_host[0, 0:33] = 1.0
    sel97_host[1, 64:97] = 1.0
    selZ_host = np.zeros((BANKW, 2), np.float32)
    selZ_host[32, 0] = 1.0
    selZ_host[96, 1] = 1.0

    if "nc" not in _PROGRAM_CACHE:
        _PROGRAM_CACHE["nc"] = build_program()
    nc = _PROGRAM_CACHE["nc"]

    in_maps = []
    for c in range(N_CORES):
        rows = slice(c * R, (c + 1) * R)
        xrow = x[rows]                                       # [512, 256]
        xr_host = np.empty((128, 4 * 256 + 2), np.float32)
        xr_host[:, 0:1024] = (
            xrow.reshape(4, 128, 2, 128).transpose(3, 0, 2, 1).reshape(128, -1))
        xr_host[:, 1024] = xsum[0:128]
        xr_host[:, 1025] = xsum[128:256]
        adjc_host = np.ascontiguousarray(
            adj[rows, :].T.reshape(JC, 128, R).transpose(1, 0, 2)
            .reshape(128, -1)).astype(ml_dtypes.bfloat16)
        in_maps.append({
            "xk": xk_host,
            "xr": xr_host,
            "wkk": wkk_host,
            "wk2": wk2_host,
            "wap": wap_host,
            "wap2": wap2_host,
            "adjc": adjc_host,
            "sel97h": sel97_host,
            "selZh": selZ_host,
            "wsb": wsb_host,
            "ws2": ws2_host,
            "xsb": xsb_host,
        })

    res = run_bass_kernel_spmd(nc, in_maps, list(range(N_CORES)))
    out = np.empty((N, OUT_FEAT), np.float32)
    for c in range(N_CORES):
        ob = res.results[c]["outB"]                          # [388, 512]
        for h in range(H):
            c0 = _col97(h)
            out[c * R:(c + 1) * R, h * D:(h + 1) * D] = ob[c0:c0 + 32].T
    return out
